# revision 19
# baseline (speedup 1.0000x reference)
"""Trainium2 Bass kernel for multi-head attention (B=2, P=2048, M=1024, N=16, H=64).

Sharding: 8 cores = 2 batches x 4 head-groups. Core c handles batch c//4,
heads [4*(c%4), 4*(c%4)+4). Each core computes its heads' attention and the
partial output projection; the host sums partials across the 4 cores of each
batch.

Device algorithm (per core, all matmuls in fp32r on the PE):
  - q^T,k^T,v^T [h', p] via projections with x^T as the moving operand,
    head-pairs concatenated to fill 128 partitions; bias added via K=1 matmul.
  - scores^T [pk, pq] per head; strictly-lower-triangular keep mask (pq < pk)
    exploited by skipping fully-masked tiles and narrowing partial ones.
  - exp on ScalarE (scale=1/8 fused in); mask applied multiplicatively after.
  - v transposed head-wise on the PE with an appended ones row, so the z
    matmul (z_aug^T = v_aug^T @ exp^T) also yields the softmax denominators.
  - denominators moved to per-partition layout via PE transpose; output
    projection runs per head into separate PSUM banks and heads are combined
    with denominator scaling via fused scalar_tensor_tensor ops.
  - The fully-masked query row P-1 (softmax of all -1e10 = uniform) is
    patched analytically on the host.
"""
import sys

import numpy as np

if "/opt/trn_rl_repo" not in sys.path:
    sys.path.insert(0, "/opt/trn_rl_repo")

import concourse.bacc as bacc
import concourse.tile as tile
from concourse import mybir
from concourse import bass_utils

B, P, M, N, H = 2, 2048, 1024, 16, 64
NCORES = 8
HPC = 4          # heads per core
NPAIRS = 2       # head pairs per core
MK = M // 128    # 8 contraction chunks for projections
PT = P // 512    # 4 free-dim tiles of 512 over sequence
PC = P // 128    # 16 partition chunks over sequence
MT = M // 512    # 2 output m-tiles

F32 = mybir.dt.float32
F32R = mybir.dt.float32r
EXP = mybir.ActivationFunctionType.Exp
MULT = mybir.AluOpType.mult
ADD = mybir.AluOpType.add

_BUILT = [None]


def _emit(nc, tc, aps, ctx):
    xT = aps["xT"]          # [1024, 2048]
    outp = aps["outp"]      # [2048, 1024]

    consts = ctx.enter_context(tc.tile_pool(name="consts", bufs=1))
    xpool = ctx.enter_context(tc.tile_pool(name="xpool", bufs=MK))
    zpool = ctx.enter_context(tc.tile_pool(name="zpool", bufs=16))

    eye = consts.tile([128, 128], F32)
    nc.sync.dma_start(eye[:], aps["eye"][:])
    mask = consts.tile([128, 128], F32)
    nc.sync.dma_start(mask[:], aps["mask"][:])
    ones32 = consts.tile([1, 512], F32)
    nc.vector.memset(ones32[:], 1.0)
    ones_row = consts.tile([1, 512], F32R)
    nc.vector.tensor_copy(ones_row[:], ones32[:])

    # x^T chunks [128 m, 2048 p]
    xsb = []
    for k in range(MK):
        xt = xpool.tile([128, 2048], F32R, tag="x")
        nc.sync.dma_start(xt[:], xT[128 * k:128 * (k + 1), :])
        xsb.append(xt)

    z_tiles = {}

    with tc.tile_pool(name="wpool", bufs=6) as wpool, \
         tc.tile_pool(name="qkpool", bufs=2) as qkpool, \
         tc.tile_pool(name="vtpool", bufs=4) as vtpool, \
         tc.tile_pool(name="vapool", bufs=40) as vapool, \
         tc.tile_pool(name="expool", bufs=6) as expool, \
         tc.tile_pool(name="ps_qkv", bufs=2, space="PSUM") as ps_qkv, \
         tc.tile_pool(name="ps_sc", bufs=2, space="PSUM") as ps_sc, \
         tc.tile_pool(name="ps_z", bufs=3, space="PSUM") as ps_z:
        # weights: per (tensor, pair) one [128, 8*128] tile of lhsT chunks
        wsb = {}
        bsb = {}
        for t in ("q", "k", "v"):
            for pr in range(NPAIRS):
                wt = wpool.tile([128, MK * 128], F32R, tag="w")
                nc.sync.dma_start(
                    wt.rearrange("p (k f) -> p k f", k=MK),
                    aps[f"w{t}"][pr].rearrange("k p f -> p k f"),
                )
                wsb[(t, pr)] = wt
                bt = consts.tile([1, 128], F32R, tag=f"b{t}{pr}")
                nc.sync.dma_start(bt[:], aps[f"b{t}"][pr])
                bsb[(t, pr)] = bt
        for pr in range(NPAIRS):
            qT = qkpool.tile([128, 2048], F32R, tag="qT")
            kT = qkpool.tile([128, 2048], F32R, tag="kT")
            for t, dest in (("q", qT), ("k", kT), ("v", None)):
                w = wsb[(t, pr)]
                for j4 in range(PT):
                    ps = ps_qkv.tile([128, 512], F32, tag="qkvps")
                    for mk in range(MK):
                        nc.tensor.matmul(
                            ps[:],
                            w[:, 128 * mk:128 * (mk + 1)],
                            xsb[mk][:, 512 * j4:512 * (j4 + 1)],
                            start=(mk == 0), stop=False,
                        )
                    nc.tensor.matmul(
                        ps[:], bsb[(t, pr)][:],
                        ones_row[:], start=False, stop=True,
                    )
                    sl = slice(512 * j4, 512 * (j4 + 1))
                    if t == "v":
                        # v^T slice + ones row, PE-transposed into v_aug
                        # chunks [128 pk, 65] (col 64 = ones for denoms)
                        for h01 in range(2):
                            vts = vtpool.tile([65, 512], F32, tag="vT")
                            nc.vector.tensor_copy(vts[64:65, :], ones32[:])
                            nc.vector.tensor_copy(
                                vts[0:64, :], ps[64 * h01:64 * (h01 + 1), :]
                            )
                            for c4 in range(4):
                                pst = ps_qkv.tile([128, 65], F32, tag="qkvps")
                                nc.tensor.transpose(
                                    pst[:], vts[:, 128 * c4:128 * (c4 + 1)],
                                    eye[0:65, 0:65],
                                )
                                va = vapool.tile([128, 65], F32R, tag="va")
                                nc.vector.tensor_copy(va[:], pst[:])
                                z_tiles[("va", pr, h01, 4 * j4 + c4)] = va
                    else:
                        nc.vector.tensor_copy(dest[:, sl], ps[:])
            # attention per head
            for h01 in range(2):
                rows = slice(64 * h01, 64 * (h01 + 1))
                for j in range(PT):
                    zps = ps_z.tile([65, 512], F32, tag="zps")
                    for i in range(PC - 1, 4 * j - 1, -1):
                        tt = i - 4 * j
                        w_ = min(512, 128 * (tt + 1))
                        sps = ps_sc.tile([128, 512], F32, tag="scps")
                        nc.tensor.matmul(
                            sps[:, :w_],
                            kT[rows, 128 * i:128 * (i + 1)],
                            qT[rows, 512 * j:512 * j + w_],
                            start=True, stop=True,
                        )
                        ex = expool.tile([128, 512], F32R, tag="ex")
                        nc.scalar.activation(
                            ex[:, :w_], sps[:, :w_], EXP, scale=0.125
                        )
                        if tt < 4:
                            nc.vector.tensor_mul(
                                ex[:, 128 * tt:w_], ex[:, 128 * tt:w_], mask[:]
                            )
                        nc.tensor.matmul(
                            zps[:, :w_],
                            z_tiles[("va", pr, h01, i)][:],
                            ex[:, :w_],
                            start=(i == PC - 1), stop=(i == 4 * j),
                        )
                    zsb = zpool.tile([65, 512], F32R, tag="z")
                    nc.vector.tensor_copy(zsb[:], zps[:])
                    if j == PT - 1:
                        # fully-masked query row P-1: denom 0 -> 1 so the
                        # reciprocal is finite (host patches the output row)
                        nc.vector.tensor_copy(zsb[64:65, 511:512], ones32[:, 0:1])
                    z_tiles[(pr, h01, j)] = zsb

    # phase B: denominators + output projection
    recips = []
    with tc.tile_pool(name="dnpool", bufs=4) as dnpool, \
         tc.tile_pool(name="opool", bufs=4) as opool, \
         tc.tile_pool(name="wopool", bufs=1) as wopool, \
         tc.tile_pool(name="ps_dn", bufs=2, space="PSUM") as ps_dn, \
         tc.tile_pool(name="ps_pr", bufs=6, space="PSUM") as ps_pr:
        wo = wopool.tile([64, HPC * 1024], F32R, tag="wo")
        nc.sync.dma_start(
            wo.rearrange("p (n f) -> p n f", n=HPC),
            aps["wo"].rearrange("n p f -> p n f"),
        )
        for u in range(HPC):
            pr, h01 = divmod(u, 2)
            dn = dnpool.tile([128, 16], F32, tag="dn")
            for j in range(PT):
                for c4 in range(4):
                    pst = ps_dn.tile([128, 65], F32, tag="dnps")
                    nc.tensor.transpose(
                        pst[:],
                        z_tiles[(pr, h01, j)][:, 128 * c4:128 * (c4 + 1)]
                        .bitcast(F32),
                        eye[0:65, 0:65],
                    )
                    nc.vector.tensor_copy(
                        dn[:, 4 * j + c4:4 * j + c4 + 1], pst[:, 64:65]
                    )
            rc = dnpool.tile([128, 16], F32, tag="rc")
            nc.vector.reciprocal(rc[:], dn[:])
            recips.append(rc)

        for ck in range(PC):
            j, c4 = divmod(ck, 4)
            for mt in range(MT):
                pss = []
                for u in range(HPC):
                    pr, h01 = divmod(u, 2)
                    pp = ps_pr.tile([128, 512], F32, tag="prps")
                    nc.tensor.matmul(
                        pp[:],
                        z_tiles[(pr, h01, j)][0:64, 128 * c4:128 * (c4 + 1)]
                        ,
                        wo[:, 1024 * u + 512 * mt:1024 * u + 512 * (mt + 1)]
                        ,
                        start=True, stop=True,
                    )
                    pss.append(pp)
                osb = opool.tile([128, 512], F32, tag="osb")
                nc.scalar.mul(osb[:], pss[0][:], mul=recips[0][:, ck:ck + 1])
                for u in range(1, HPC):
                    nc.vector.scalar_tensor_tensor(
                        osb[:], pss[u][:], recips[u][:, ck:ck + 1], osb[:],
                        MULT, ADD,
                    )
                nc.sync.dma_start(
                    outp[128 * ck:128 * (ck + 1), 512 * mt:512 * (mt + 1)],
                    osb[:],
                )


def _build():
    if _BUILT[0] is not None:
        return _BUILT[0]
    from contextlib import ExitStack

    nc = bacc.Bacc("TRN2", target_bir_lowering=False, debug=False)
    aps = {
        "xT": nc.dram_tensor("xT", [M, P], F32R, kind="ExternalInput").ap(),
        "wq": nc.dram_tensor("wq", [NPAIRS, MK, 128, 128], F32R,
                             kind="ExternalInput").ap(),
        "wk": nc.dram_tensor("wk", [NPAIRS, MK, 128, 128], F32R,
                             kind="ExternalInput").ap(),
        "wv": nc.dram_tensor("wv", [NPAIRS, MK, 128, 128], F32R,
                             kind="ExternalInput").ap(),
        "wo": nc.dram_tensor("wo", [HPC, 64, 1024], F32R,
                             kind="ExternalInput").ap(),
        "bq": nc.dram_tensor("bq", [NPAIRS, 1, 128], F32R,
                             kind="ExternalInput").ap(),
        "bk": nc.dram_tensor("bk", [NPAIRS, 1, 128], F32R,
                             kind="ExternalInput").ap(),
        "bv": nc.dram_tensor("bv", [NPAIRS, 1, 128], F32R,
                             kind="ExternalInput").ap(),
        "eye": nc.dram_tensor("eye", [128, 128], F32,
                              kind="ExternalInput").ap(),
        "mask": nc.dram_tensor("mask", [128, 128], F32,
                               kind="ExternalInput").ap(),
        "outp": nc.dram_tensor("outp", [P, M], F32, kind="ExternalOutput").ap(),
    }
    with tile.TileContext(nc) as tc:
        with ExitStack() as ctx:
            _emit(nc, tc, aps, ctx)
    nc.compile()
    _BUILT[0] = nc
    return nc


def _host_inputs(x, kq, kk, kv, ko, bq, bk, bv):
    xT = np.ascontiguousarray(x.transpose(0, 2, 1))  # [B, M, P]
    eye = np.eye(128, dtype=np.float32)
    # keep iff pq < pk; block mask[r(pk), c(pq)] = 1 if c < r
    mask = np.tril(np.ones((128, 128), np.float32), k=-1)
    in_maps = []
    for c in range(NCORES):
        b, k4 = divmod(c, 4)
        heads = [4 * k4 + i for i in range(HPC)]

        def pairw(kern):
            # [NPAIRS, MK, 128, 128] lhsT chunks
            out = np.empty((NPAIRS, MK, 128, 128), np.float32)
            for pr in range(NPAIRS):
                pairm = np.concatenate(
                    [kern[heads[2 * pr]], kern[heads[2 * pr + 1]]], axis=1
                )  # [1024, 128]
                out[pr] = pairm.reshape(MK, 128, 128)
            return out

        def pairb(bias):
            out = np.empty((NPAIRS, 1, 128), np.float32)
            for pr in range(NPAIRS):
                out[pr, 0] = np.concatenate(
                    [bias[heads[2 * pr]], bias[heads[2 * pr + 1]]]
                )
            return out

        in_maps.append({
            "xT": xT[b],
            "wq": pairw(kq), "wk": pairw(kk), "wv": pairw(kv),
            "wo": np.ascontiguousarray(ko[heads]),
            "bq": pairb(bq), "bk": pairb(bk), "bv": pairb(bv),
            "eye": eye, "mask": mask,
        })
    return in_maps


def kernel(x, kernel_query, kernel_key, kernel_value, kernel_out,
           bias_query, bias_key, bias_value, bias_out, _trace=False):
    x = np.asarray(x, np.float32)
    kq = np.asarray(kernel_query, np.float32)
    kk = np.asarray(kernel_key, np.float32)
    kv = np.asarray(kernel_value, np.float32)
    ko = np.asarray(kernel_out, np.float32)
    bq = np.asarray(bias_query, np.float32)
    bk = np.asarray(bias_key, np.float32)
    bv = np.asarray(bias_value, np.float32)
    bo = np.asarray(bias_out, np.float32)

    nc = _build()
    in_maps = _host_inputs(x, kq, kk, kv, ko, bq, bk, bv)
    res = bass_utils.run_bass_kernel_spmd(
        nc, in_maps, core_ids=list(range(NCORES)), trace=_trace
    )
    out = np.zeros((B, P, M), np.float32)
    for c in range(NCORES):
        out[c // 4] += res.results[c]["outp"]
    out += bo[None, None, :]

    # patch fully-masked query row P-1: uniform attention = mean_k v
    for b in range(B):
        xbar = x[b].mean(axis=0, dtype=np.float64)  # [M]
        row = np.zeros(M, np.float64)
        for n in range(N):
            zrow = xbar @ kv[n].astype(np.float64) + bv[n].astype(np.float64)
            row += zrow @ ko[n].astype(np.float64)
        out[b, P - 1, :] = (row + bo.astype(np.float64)).astype(np.float32)

    if _trace:
        kernel._last_result = res
    return out


# revision 22
# speedup vs baseline: 1.3493x; 1.3493x over previous
"""Trainium2 Bass kernel for multi-head attention (B=2, P=2048, M=1024, N=16, H=64).

Sharding: 8 cores = 2 batches x 4 head-groups. Core c handles batch c//4,
heads [4*(c%4), 4*(c%4)+4). Each core computes its heads' attention and the
partial output projection; the host sums partials across the 4 cores of each
batch.

Device algorithm (per core; matmul dtype selectable bf16/fp32r):
  - q^T,k^T,v^T [h', p] via projections with x^T as the moving operand,
    head-pairs concatenated to fill 128 partitions; bias added via K=1 matmul.
    One weight load feeds 4 accumulating p-tiles (LDWEIGHTS amortized).
  - scores^T [pk, pq] per head; strictly-lower-triangular keep mask (pq < pk)
    exploited by skipping fully-masked tiles and narrowing partial ones.
  - exp on ScalarE (scale=1/8 fused in); mask applied multiplicatively after.
  - v transposed head-wise on the PE with an appended ones row, so the z
    matmul (z_aug^T = v_aug^T @ exp^T) also yields the softmax denominators
    (kept in fp32 regardless of matmul dtype).
  - denominators moved to per-partition layout via PE transpose; output
    projection runs per head into separate PSUM banks and heads are combined
    with denominator scaling via fused scalar_tensor_tensor ops.
  - The fully-masked query row P-1 (softmax of all -1e10 = uniform) is
    patched analytically on the host.
"""
import os
import sys

import numpy as np

if "/opt/trn_rl_repo" not in sys.path:
    sys.path.insert(0, "/opt/trn_rl_repo")

import concourse.bacc as bacc
import concourse.tile as tile
from concourse import mybir
from concourse import bass_utils
import ml_dtypes

B, P, M, N, H = 2, 2048, 1024, 16, 64
NCORES = 8
HPC = 4          # heads per core
NPAIRS = 2       # head pairs per core
MK = M // 128    # 8 contraction chunks for projections
PT = P // 512    # 4 free-dim tiles of 512 over sequence
PC = P // 128    # 16 partition chunks over sequence
MT = M // 512    # 2 output m-tiles

F32 = mybir.dt.float32
F32R = mybir.dt.float32r
BF16 = mybir.dt.bfloat16
EXP = mybir.ActivationFunctionType.Exp
MULT = mybir.AluOpType.mult
ADD = mybir.AluOpType.add

DT_MODE = os.environ.get("KERNEL_DT", "bf16")   # "bf16" | "f32r"
DT_MM = BF16 if DT_MODE == "bf16" else F32R
NP_MM = ml_dtypes.bfloat16 if DT_MODE == "bf16" else np.float32

_BUILT = {}


def _emit(nc, tc, aps, ctx):
    xT = aps["xT"]          # [1024, 2048]
    outp = aps["outp"]      # [2048, 1024]
    dual_z = DT_MODE == "bf16"   # separate bf16 lhsT copy + f32 denominators

    consts = ctx.enter_context(tc.tile_pool(name="consts", bufs=1))
    xpool = ctx.enter_context(tc.tile_pool(name="xpool", bufs=MK))
    zpool = ctx.enter_context(tc.tile_pool(name="zpool", bufs=16))
    zbpool = ctx.enter_context(tc.tile_pool(name="zbpool", bufs=16)) \
        if dual_z else None

    eye = consts.tile([128, 128], F32)
    nc.sync.dma_start(eye[:], aps["eye"][:])
    mask = consts.tile([128, 128], DT_MM)
    nc.sync.dma_start(mask[:], aps["mask"][:])
    ones32 = consts.tile([1, 512], F32)
    nc.vector.memset(ones32[:], 1.0)
    if DT_MODE == "bf16":
        ones_mm = consts.tile([1, 512], BF16)
        nc.vector.memset(ones_mm[:], 1.0)
    else:
        ones_mm = consts.tile([1, 512], F32R)
        nc.vector.tensor_copy(ones_mm[:], ones32[:])

    # x^T chunks [128 m, 2048 p]
    xsb = []
    for k in range(MK):
        xt = xpool.tile([128, 2048], DT_MM, tag="x")
        nc.sync.dma_start(xt[:], xT[128 * k:128 * (k + 1), :])
        xsb.append(xt)

    z_tiles = {}

    with tc.tile_pool(name="wpool", bufs=6) as wpool, \
         tc.tile_pool(name="qkpool", bufs=2) as qkpool, \
         tc.tile_pool(name="vtpool", bufs=4) as vtpool, \
         tc.tile_pool(name="vapool", bufs=40) as vapool, \
         tc.tile_pool(name="expool", bufs=(8 if DT_MODE == "bf16" else 5)) as expool, \
         tc.tile_pool(name="ps_qkv", bufs=4, space="PSUM") as ps_qkv, \
         tc.tile_pool(name="ps_sc", bufs=2, space="PSUM") as ps_sc, \
         tc.tile_pool(name="ps_z", bufs=2, space="PSUM") as ps_z:
        # weights: per (tensor, pair) one [128, 8*128] tile of lhsT chunks
        wsb = {}
        bsb = {}
        for t in ("q", "k", "v"):
            for pr in range(NPAIRS):
                wt = wpool.tile([128, MK * 128], DT_MM, tag="w")
                nc.sync.dma_start(
                    wt.rearrange("p (k f) -> p k f", k=MK),
                    aps[f"w{t}"][pr].rearrange("k p f -> p k f"),
                )
                wsb[(t, pr)] = wt
                bt = consts.tile([1, 128], DT_MM, tag=f"b{t}{pr}")
                nc.sync.dma_start(bt[:], aps[f"b{t}"][pr])
                bsb[(t, pr)] = bt
        for pr in range(NPAIRS):
            qT = qkpool.tile([128, 2048], DT_MM, tag="qT")
            kT = qkpool.tile([128, 2048], DT_MM, tag="kT")
            for t, dest in (("q", qT), ("k", kT), ("v", None)):
                w = wsb[(t, pr)]
                # one LDWEIGHTS per m-chunk feeds 4 accumulating p-tiles
                pss = [ps_qkv.tile([128, 512], F32, tag="qkvps",
                                   name=f"qkvps_{t}{pr}{j4}")
                       for j4 in range(PT)]
                for mk in range(MK):
                    for j4 in range(PT):
                        nc.tensor.matmul(
                            pss[j4][:],
                            w[:, 128 * mk:128 * (mk + 1)],
                            xsb[mk][:, 512 * j4:512 * (j4 + 1)],
                            start=(mk == 0), stop=False,
                        )
                for j4 in range(PT):
                    nc.tensor.matmul(
                        pss[j4][:], bsb[(t, pr)][:],
                        ones_mm[:], start=False, stop=True,
                    )
                for j4 in range(PT):
                    ps = pss[j4]
                    sl = slice(512 * j4, 512 * (j4 + 1))
                    if t == "v":
                        # v^T slice + ones row, PE-transposed into v_aug
                        # chunks [128 pk, 65] (col 64 = ones for denoms)
                        for h01 in range(2):
                            vts = vtpool.tile([65, 512], F32, tag="vT")
                            nc.vector.tensor_copy(vts[64:65, :], ones32[:])
                            nc.vector.tensor_copy(
                                vts[0:64, :], ps[64 * h01:64 * (h01 + 1), :]
                            )
                            for c4 in range(4):
                                pst = ps_qkv.tile([128, 65], F32,
                                                  tag="qkvps")
                                nc.tensor.transpose(
                                    pst[:], vts[:, 128 * c4:128 * (c4 + 1)],
                                    eye[0:65, 0:65],
                                )
                                va = vapool.tile([128, 65], DT_MM, tag="va")
                                nc.vector.tensor_copy(va[:], pst[:])
                                z_tiles[("va", pr, h01, 4 * j4 + c4)] = va
                    else:
                        nc.vector.tensor_copy(dest[:, sl], ps[:])
            # attention per head
            for h01 in range(2):
                rows = slice(64 * h01, 64 * (h01 + 1))
                for j in range(PT):
                    zps = ps_z.tile([65, 512], F32, tag="zps")
                    for i in range(PC - 1, 4 * j - 1, -1):
                        tt = i - 4 * j
                        w_ = min(512, 128 * (tt + 1))
                        sps = ps_sc.tile([128, 512], F32, tag="scps")
                        nc.tensor.matmul(
                            sps[:, :w_],
                            kT[rows, 128 * i:128 * (i + 1)],
                            qT[rows, 512 * j:512 * j + w_],
                            start=True, stop=True,
                        )
                        ex = expool.tile([128, 512], DT_MM, tag="ex")
                        nc.scalar.activation(
                            ex[:, :w_], sps[:, :w_], EXP, scale=0.125
                        )
                        if tt < 4:
                            nc.vector.tensor_mul(
                                ex[:, 128 * tt:w_], ex[:, 128 * tt:w_], mask[:]
                            )
                        nc.tensor.matmul(
                            zps[:, :w_],
                            z_tiles[("va", pr, h01, i)][:],
                            ex[:, :w_],
                            start=(i == PC - 1), stop=(i == 4 * j),
                        )
                    zsb = zpool.tile([65, 512], F32R, tag="z")
                    nc.vector.tensor_copy(zsb[:], zps[:])
                    if j == PT - 1:
                        # fully-masked query row P-1: denom 0 -> 1 so the
                        # reciprocal is finite (host patches the output row)
                        nc.vector.tensor_copy(
                            zsb[64:65, 511:512], ones32[:, 0:1]
                        )
                    z_tiles[(pr, h01, j)] = zsb
                    if dual_z:
                        zbf = zbpool.tile([64, 512], BF16, tag="zb")
                        nc.vector.tensor_copy(zbf[:], zps[0:64, :])
                        z_tiles[("zb", pr, h01, j)] = zbf

    # phase B: denominators + output projection
    recips = []
    with tc.tile_pool(name="dnpool", bufs=8) as dnpool, \
         tc.tile_pool(name="opool", bufs=4) as opool, \
         tc.tile_pool(name="wopool", bufs=1) as wopool, \
         tc.tile_pool(name="ps_dn", bufs=2, space="PSUM") as ps_dn, \
         tc.tile_pool(name="ps_pr", bufs=6, space="PSUM") as ps_pr:
        wo = wopool.tile([64, HPC * 1024], DT_MM, tag="wo")
        nc.sync.dma_start(
            wo.rearrange("p (n f) -> p n f", n=HPC),
            aps["wo"].rearrange("n p f -> p n f"),
        )
        for u in range(HPC):
            pr, h01 = divmod(u, 2)
            dn = dnpool.tile([128, 16], F32, tag="dn")
            for j in range(PT):
                for c4 in range(4):
                    pst = ps_dn.tile([128, 65], F32, tag="dnps")
                    nc.tensor.transpose(
                        pst[:],
                        z_tiles[(pr, h01, j)][:, 128 * c4:128 * (c4 + 1)]
                        .bitcast(F32),
                        eye[0:65, 0:65],
                    )
                    nc.vector.tensor_copy(
                        dn[:, 4 * j + c4:4 * j + c4 + 1], pst[:, 64:65]
                    )
            rc = dnpool.tile([128, 16], F32, tag="rc")
            nc.vector.reciprocal(rc[:], dn[:])
            recips.append(rc)

        for ck in range(PC):
            j, c4 = divmod(ck, 4)
            for mt in range(MT):
                pss = []
                for u in range(HPC):
                    pr, h01 = divmod(u, 2)
                    if dual_z:
                        lhs = z_tiles[("zb", pr, h01, j)][
                            0:64, 128 * c4:128 * (c4 + 1)]
                    else:
                        lhs = z_tiles[(pr, h01, j)][
                            0:64, 128 * c4:128 * (c4 + 1)]
                    pp = ps_pr.tile([128, 512], F32, tag="prps")
                    nc.tensor.matmul(
                        pp[:], lhs,
                        wo[:, 1024 * u + 512 * mt:1024 * u + 512 * (mt + 1)],
                        start=True, stop=True,
                    )
                    pss.append(pp)
                osb = opool.tile([128, 512], F32, tag="osb")
                nc.scalar.mul(osb[:], pss[0][:], mul=recips[0][:, ck:ck + 1])
                for u in range(1, HPC):
                    nc.vector.scalar_tensor_tensor(
                        osb[:], pss[u][:], recips[u][:, ck:ck + 1], osb[:],
                        MULT, ADD,
                    )
                nc.sync.dma_start(
                    outp[128 * ck:128 * (ck + 1), 512 * mt:512 * (mt + 1)],
                    osb[:],
                )


def _build():
    if DT_MODE in _BUILT:
        return _BUILT[DT_MODE]
    from contextlib import ExitStack

    nc = bacc.Bacc("TRN2", target_bir_lowering=False, debug=False)
    aps = {
        "xT": nc.dram_tensor("xT", [M, P], DT_MM, kind="ExternalInput").ap(),
        "wq": nc.dram_tensor("wq", [NPAIRS, MK, 128, 128], DT_MM,
                             kind="ExternalInput").ap(),
        "wk": nc.dram_tensor("wk", [NPAIRS, MK, 128, 128], DT_MM,
                             kind="ExternalInput").ap(),
        "wv": nc.dram_tensor("wv", [NPAIRS, MK, 128, 128], DT_MM,
                             kind="ExternalInput").ap(),
        "wo": nc.dram_tensor("wo", [HPC, 64, 1024], DT_MM,
                             kind="ExternalInput").ap(),
        "bq": nc.dram_tensor("bq", [NPAIRS, 1, 128], DT_MM,
                             kind="ExternalInput").ap(),
        "bk": nc.dram_tensor("bk", [NPAIRS, 1, 128], DT_MM,
                             kind="ExternalInput").ap(),
        "bv": nc.dram_tensor("bv", [NPAIRS, 1, 128], DT_MM,
                             kind="ExternalInput").ap(),
        "eye": nc.dram_tensor("eye", [128, 128], F32,
                              kind="ExternalInput").ap(),
        "mask": nc.dram_tensor("mask", [128, 128], DT_MM,
                               kind="ExternalInput").ap(),
        "outp": nc.dram_tensor("outp", [P, M], F32, kind="ExternalOutput").ap(),
    }
    with tile.TileContext(nc) as tc:
        with ExitStack() as ctx:
            _emit(nc, tc, aps, ctx)
    nc.compile()
    _BUILT[DT_MODE] = nc
    return nc


def _host_inputs(x, kq, kk, kv, ko, bq, bk, bv):
    xT = np.ascontiguousarray(x.transpose(0, 2, 1)).astype(NP_MM)  # [B, M, P]
    eye = np.eye(128, dtype=np.float32)
    # keep iff pq < pk; block mask[r(pk), c(pq)] = 1 if c < r
    mask = np.tril(np.ones((128, 128), np.float32), k=-1).astype(NP_MM)
    in_maps = []
    for c in range(NCORES):
        b, k4 = divmod(c, 4)
        heads = [4 * k4 + i for i in range(HPC)]

        def pairw(kern):
            # [NPAIRS, MK, 128, 128] lhsT chunks
            out = np.empty((NPAIRS, MK, 128, 128), NP_MM)
            for pr in range(NPAIRS):
                pairm = np.concatenate(
                    [kern[heads[2 * pr]], kern[heads[2 * pr + 1]]], axis=1
                )  # [1024, 128]
                out[pr] = pairm.reshape(MK, 128, 128).astype(NP_MM)
            return out

        def pairb(bias):
            out = np.empty((NPAIRS, 1, 128), NP_MM)
            for pr in range(NPAIRS):
                out[pr, 0] = np.concatenate(
                    [bias[heads[2 * pr]], bias[heads[2 * pr + 1]]]
                ).astype(NP_MM)
            return out

        in_maps.append({
            "xT": xT[b],
            "wq": pairw(kq), "wk": pairw(kk), "wv": pairw(kv),
            "wo": np.ascontiguousarray(ko[heads]).astype(NP_MM),
            "bq": pairb(bq), "bk": pairb(bk), "bv": pairb(bv),
            "eye": eye, "mask": mask,
        })
    return in_maps


def kernel(x, kernel_query, kernel_key, kernel_value, kernel_out,
           bias_query, bias_key, bias_value, bias_out, _trace=False):
    x = np.asarray(x, np.float32)
    kq = np.asarray(kernel_query, np.float32)
    kk = np.asarray(kernel_key, np.float32)
    kv = np.asarray(kernel_value, np.float32)
    ko = np.asarray(kernel_out, np.float32)
    bq = np.asarray(bias_query, np.float32)
    bk = np.asarray(bias_key, np.float32)
    bv = np.asarray(bias_value, np.float32)
    bo = np.asarray(bias_out, np.float32)

    nc = _build()
    in_maps = _host_inputs(x, kq, kk, kv, ko, bq, bk, bv)
    res = bass_utils.run_bass_kernel_spmd(
        nc, in_maps, core_ids=list(range(NCORES)), trace=_trace
    )
    out = np.zeros((B, P, M), np.float32)
    for c in range(NCORES):
        out[c // 4] += res.results[c]["outp"]
    out += bo[None, None, :]

    # patch fully-masked query row P-1: uniform attention = mean_k v
    for b in range(B):
        xbar = x[b].mean(axis=0, dtype=np.float64)  # [M]
        row = np.zeros(M, np.float64)
        for n in range(N):
            zrow = xbar @ kv[n].astype(np.float64) + bv[n].astype(np.float64)
            row += zrow @ ko[n].astype(np.float64)
        out[b, P - 1, :] = (row + bo.astype(np.float64)).astype(np.float32)

    if _trace:
        kernel._last_result = res
    return out


# revision 25
# speedup vs baseline: 1.3854x; 1.0268x over previous
"""Trainium2 Bass kernel for multi-head attention (B=2, P=2048, M=1024, N=16, H=64).

Sharding: 8 cores = 2 batches x 4 head-groups. Core c handles batch c//4,
heads [4*(c%4), 4*(c%4)+4). Each core computes its heads' attention and the
partial output projection; the host sums partials across the 4 cores of each
batch.

Device algorithm (per core; matmul dtype selectable bf16/fp32r):
  - q^T,k^T,v^T [h', p] via projections with x^T as the moving operand,
    head-pairs concatenated to fill 128 partitions; bias added via K=1 matmul.
    One weight load feeds 4 accumulating p-tiles (LDWEIGHTS amortized).
  - scores^T [pk, pq] per head; strictly-lower-triangular keep mask (pq < pk)
    exploited by skipping fully-masked tiles and narrowing partial ones.
    Two pk-chunks of scores land in one [128,1024] PSUM tile so a single
    ScalarE exp instruction covers both (amortizes ACT fixed overhead).
  - v transposed head-wise on the PE with an appended ones row, so the z
    matmul (z_aug^T = v_aug^T @ exp^T) also yields the softmax denominators.
  - z_aug^T is PE-transposed to [pq, h] layout where the denominator is a
    per-partition scalar: reciprocal + tensor_scalar normalize, then
    PE-transposed back and head-pairs packed to K=128 for the output
    projection, which accumulates both pairs in PSUM. This per-unit work is
    interleaved with the attention stream to keep the PE fed while ScalarE
    runs exp.
  - The fully-masked query row P-1 (softmax of all -1e10 = uniform) is
    patched analytically on the host.
"""
import os
import sys

import numpy as np

if "/opt/trn_rl_repo" not in sys.path:
    sys.path.insert(0, "/opt/trn_rl_repo")

import concourse.bacc as bacc
import concourse.tile as tile
from concourse import mybir
from concourse import bass_utils
import ml_dtypes

B, P, M, N, H = 2, 2048, 1024, 16, 64
NCORES = 8
HPC = 4          # heads per core
NPAIRS = 2       # head pairs per core
MK = M // 128    # 8 contraction chunks for projections
PT = P // 512    # 4 free-dim tiles of 512 over sequence
PC = P // 128    # 16 partition chunks over sequence
MT = M // 512    # 2 output m-tiles

F32 = mybir.dt.float32
F32R = mybir.dt.float32r
BF16 = mybir.dt.bfloat16
EXP = mybir.ActivationFunctionType.Exp
MULT = mybir.AluOpType.mult

DT_MODE = os.environ.get("KERNEL_DT", "bf16")   # "bf16" | "f32r"
DT_MM = BF16 if DT_MODE == "bf16" else F32R
NP_MM = ml_dtypes.bfloat16 if DT_MODE == "bf16" else np.float32

_BUILT = {}


def _emit(nc, tc, aps, ctx):
    xT = aps["xT"]          # [1024, 2048]
    outp = aps["outp"]      # [2048, 1024]

    consts = ctx.enter_context(tc.tile_pool(name="consts", bufs=1))
    xpool = ctx.enter_context(tc.tile_pool(name="xpool", bufs=MK))
    qkpool = ctx.enter_context(tc.tile_pool(name="qkpool", bufs=2))
    vapool = ctx.enter_context(tc.tile_pool(name="vapool", bufs=68))
    zppool = ctx.enter_context(tc.tile_pool(name="zppool", bufs=16))

    eye = consts.tile([128, 128], F32)
    nc.sync.dma_start(eye[:], aps["eye"][:])
    mask = consts.tile([128, 128], DT_MM)
    nc.sync.dma_start(mask[:], aps["mask"][:])
    ones32 = consts.tile([1, 512], F32)
    nc.vector.memset(ones32[:], 1.0)
    if DT_MODE == "bf16":
        ones_mm = consts.tile([1, 512], BF16)
        nc.vector.memset(ones_mm[:], 1.0)
    else:
        ones_mm = consts.tile([1, 512], F32R)
        nc.vector.tensor_copy(ones_mm[:], ones32[:])
    wos = []
    for pr in range(NPAIRS):
        wot = consts.tile([128, 1024], DT_MM, tag=f"wo{pr}", name=f"wo{pr}")
        nc.sync.dma_start(wot[:], aps["wo"][pr])
        wos.append(wot)

    # x^T chunks [128 m, 2048 p]
    xsb = []
    for k in range(MK):
        xt = xpool.tile([128, 2048], DT_MM, tag="x")
        nc.sync.dma_start(xt[:], xT[128 * k:128 * (k + 1), :])
        xsb.append(xt)

    tiles = {}
    qts, kts = {}, {}

    # ---------------- QKV projections ----------------
    with tc.tile_pool(name="wpool", bufs=6) as wpool, \
         tc.tile_pool(name="vtpool", bufs=4) as vtpool, \
         tc.tile_pool(name="ps_qkv", bufs=4, space="PSUM") as ps_qkv:
        wsb = {}
        bsb = {}
        for t in ("q", "k", "v"):
            for pr in range(NPAIRS):
                wt = wpool.tile([128, MK * 128], DT_MM, tag="w")
                nc.sync.dma_start(
                    wt.rearrange("p (k f) -> p k f", k=MK),
                    aps[f"w{t}"][pr].rearrange("k p f -> p k f"),
                )
                wsb[(t, pr)] = wt
                bt = consts.tile([1, 128], DT_MM, tag=f"b{t}{pr}")
                nc.sync.dma_start(bt[:], aps[f"b{t}"][pr])
                bsb[(t, pr)] = bt
        for pr in range(NPAIRS):
            qT = qkpool.tile([128, 2048], DT_MM, tag="qT", name=f"qT{pr}")
            kT = qkpool.tile([128, 2048], DT_MM, tag="kT", name=f"kT{pr}")
            qts[pr], kts[pr] = qT, kT
            for t, dest in (("q", qT), ("k", kT), ("v", None)):
                w = wsb[(t, pr)]
                # one LDWEIGHTS per m-chunk feeds 4 accumulating p-tiles
                pss = [ps_qkv.tile([128, 512], F32, tag="qkvps",
                                   name=f"qkvps_{t}{pr}{j4}")
                       for j4 in range(PT)]
                for mk in range(MK):
                    for j4 in range(PT):
                        nc.tensor.matmul(
                            pss[j4][:],
                            w[:, 128 * mk:128 * (mk + 1)],
                            xsb[mk][:, 512 * j4:512 * (j4 + 1)],
                            start=(mk == 0), stop=False,
                        )
                for j4 in range(PT):
                    nc.tensor.matmul(
                        pss[j4][:], bsb[(t, pr)][:],
                        ones_mm[:], start=False, stop=True,
                    )
                for j4 in range(PT):
                    ps = pss[j4]
                    if t == "v":
                        # v^T slice + ones row, PE-transposed into v_aug
                        # chunks [128 pk, 65] (col 64 = ones for denoms)
                        for h01 in range(2):
                            vts = vtpool.tile([65, 512], F32, tag="vT")
                            nc.vector.tensor_copy(vts[64:65, :], ones32[:])
                            nc.vector.tensor_copy(
                                vts[0:64, :], ps[64 * h01:64 * (h01 + 1), :]
                            )
                            for c4 in range(4):
                                pst = ps_qkv.tile([128, 65], F32,
                                                  tag="qkvps")
                                nc.tensor.transpose(
                                    pst[:], vts[:, 128 * c4:128 * (c4 + 1)],
                                    eye[0:65, 0:65],
                                )
                                va = vapool.tile([128, 65], DT_MM, tag="va")
                                nc.vector.tensor_copy(va[:], pst[:])
                                tiles[("va", pr, h01, 4 * j4 + c4)] = va
                    else:
                        nc.vector.tensor_copy(
                            dest[:, 512 * j4:512 * (j4 + 1)], ps[:]
                        )

    # ------- attention + normalization + projection, pipelined per j -------
    with tc.tile_pool(name="expool", bufs=6) as expool, \
         tc.tile_pool(name="zsbpool", bufs=6) as zsbpool, \
         tc.tile_pool(name="znpool", bufs=6) as znpool, \
         tc.tile_pool(name="rcpool", bufs=8) as rcpool, \
         tc.tile_pool(name="opool", bufs=4) as opool, \
         tc.tile_pool(name="ps_sc", bufs=2, space="PSUM") as ps_sc, \
         tc.tile_pool(name="ps_z", bufs=1, space="PSUM") as ps_z, \
         tc.tile_pool(name="ps_t", bufs=3, space="PSUM") as ps_t:
        ps_pr = ps_t
        for j in range(PT):
            for pr in range(NPAIRS):
                for c4 in range(4):
                    tiles[("zp", pr, 4 * j + c4)] = zppool.tile(
                        [128, 128], DT_MM, tag="zp",
                        name=f"zp{pr}_{4 * j + c4}")
            for u in range(HPC):
                pr, h01 = divmod(u, 2)
                qT, kT = qts[pr], kts[pr]
                rows = slice(64 * h01, 64 * (h01 + 1))
                zps = ps_z.tile([65, 512], F32, tag="zps")
                ilist = list(range(PC - 1, 4 * j - 1, -1))
                ipairs = [(ilist[a], ilist[a + 1])
                          for a in range(0, len(ilist), 2)]
                for a, (i1, i2) in enumerate(ipairs):
                    w1 = min(512, 128 * (i1 - 4 * j + 1))
                    w2 = min(512, 128 * (i2 - 4 * j + 1))
                    sps = ps_sc.tile([128, 1024], F32, tag="scps")
                    nc.tensor.matmul(
                        sps[:, :w1],
                        kT[rows, 128 * i1:128 * (i1 + 1)],
                        qT[rows, 512 * j:512 * j + w1],
                        start=True, stop=True,
                    )
                    nc.tensor.matmul(
                        sps[:, 512:512 + w2],
                        kT[rows, 128 * i2:128 * (i2 + 1)],
                        qT[rows, 512 * j:512 * j + w2],
                        start=True, stop=True,
                    )
                    ex = expool.tile([128, 1024], DT_MM, tag="ex")
                    if w1 == 512:
                        nc.scalar.activation(
                            ex[:, :512 + w2], sps[:, :512 + w2], EXP,
                            scale=0.125,
                        )
                    else:
                        nc.scalar.activation(
                            ex[:, :w1], sps[:, :w1], EXP, scale=0.125
                        )
                        nc.scalar.activation(
                            ex[:, 512:512 + w2], sps[:, 512:512 + w2], EXP,
                            scale=0.125,
                        )
                    for i_, off, w_ in ((i1, 0, w1), (i2, 512, w2)):
                        tt = i_ - 4 * j
                        if tt < 4:
                            nc.vector.tensor_mul(
                                ex[:, off + 128 * tt:off + w_],
                                ex[:, off + 128 * tt:off + w_], mask[:]
                            )
                    nc.tensor.matmul(
                        zps[:, :w1], tiles[("va", pr, h01, i1)][:],
                        ex[:, :w1],
                        start=(a == 0), stop=False,
                    )
                    nc.tensor.matmul(
                        zps[:, :w2], tiles[("va", pr, h01, i2)][:],
                        ex[:, 512:512 + w2],
                        start=False, stop=(a == len(ipairs) - 1),
                    )
                zsb = zsbpool.tile([65, 512], F32, tag="z")
                nc.vector.tensor_copy(zsb[:], zps[:])
                if j == PT - 1:
                    # fully-masked query row P-1: denom 0 -> 1 so the
                    # reciprocal is finite (host patches the output row)
                    nc.vector.tensor_copy(zsb[64:65, 511:512], ones32[:, 0:1])
                # normalize in pq-space and pack into zpair
                for c4 in range(4):
                    pst1 = ps_t.tile([128, 65], F32, tag="tps")
                    nc.tensor.transpose(
                        pst1[:], zsb[:, 128 * c4:128 * (c4 + 1)],
                        eye[0:65, 0:65],
                    )
                    rcol = rcpool.tile([128, 1], F32, tag="rc")
                    nc.vector.reciprocal(rcol[:], pst1[:, 64:65])
                    zn = znpool.tile([128, 64], F32, tag="zn")
                    nc.vector.tensor_scalar_mul(zn[:], pst1[:, 0:64], rcol[:])
                    pst2 = ps_t.tile([64, 128], F32, tag="tps")
                    nc.tensor.transpose(pst2[:], zn[:], eye[:])
                    nc.vector.tensor_copy(
                        tiles[("zp", pr, 4 * j + c4)][rows, :], pst2[:]
                    )
            # projections for this j's pq chunks (both pairs accumulate)
            for c4 in range(4):
                ck = 4 * j + c4
                for mt in range(MT):
                    pp = ps_pr.tile([128, 512], F32, tag="tps", name=f"prps{ck}_{mt}")
                    nc.tensor.matmul(
                        pp[:], tiles[("zp", 0, ck)][:],
                        wos[0][:, 512 * mt:512 * (mt + 1)],
                        start=True, stop=False,
                    )
                    nc.tensor.matmul(
                        pp[:], tiles[("zp", 1, ck)][:],
                        wos[1][:, 512 * mt:512 * (mt + 1)],
                        start=False, stop=True,
                    )
                    osb = opool.tile([128, 512], F32, tag="osb")
                    nc.scalar.copy(osb[:], pp[:])
                    nc.sync.dma_start(
                        outp[128 * ck:128 * (ck + 1),
                             512 * mt:512 * (mt + 1)],
                        osb[:],
                    )


def _build():
    if DT_MODE in _BUILT:
        return _BUILT[DT_MODE]
    from contextlib import ExitStack

    nc = bacc.Bacc("TRN2", target_bir_lowering=False, debug=False)
    aps = {
        "xT": nc.dram_tensor("xT", [M, P], DT_MM, kind="ExternalInput").ap(),
        "wq": nc.dram_tensor("wq", [NPAIRS, MK, 128, 128], DT_MM,
                             kind="ExternalInput").ap(),
        "wk": nc.dram_tensor("wk", [NPAIRS, MK, 128, 128], DT_MM,
                             kind="ExternalInput").ap(),
        "wv": nc.dram_tensor("wv", [NPAIRS, MK, 128, 128], DT_MM,
                             kind="ExternalInput").ap(),
        "wo": nc.dram_tensor("wo", [NPAIRS, 128, 1024], DT_MM,
                             kind="ExternalInput").ap(),
        "bq": nc.dram_tensor("bq", [NPAIRS, 1, 128], DT_MM,
                             kind="ExternalInput").ap(),
        "bk": nc.dram_tensor("bk", [NPAIRS, 1, 128], DT_MM,
                             kind="ExternalInput").ap(),
        "bv": nc.dram_tensor("bv", [NPAIRS, 1, 128], DT_MM,
                             kind="ExternalInput").ap(),
        "eye": nc.dram_tensor("eye", [128, 128], F32,
                              kind="ExternalInput").ap(),
        "mask": nc.dram_tensor("mask", [128, 128], DT_MM,
                               kind="ExternalInput").ap(),
        "outp": nc.dram_tensor("outp", [P, M], F32, kind="ExternalOutput").ap(),
    }
    with tile.TileContext(nc) as tc:
        with ExitStack() as ctx:
            _emit(nc, tc, aps, ctx)
    nc.compile()
    _BUILT[DT_MODE] = nc
    return nc


def _host_inputs(x, kq, kk, kv, ko, bq, bk, bv):
    xT = np.ascontiguousarray(x.transpose(0, 2, 1)).astype(NP_MM)  # [B, M, P]
    eye = np.eye(128, dtype=np.float32)
    # keep iff pq < pk; block mask[r(pk), c(pq)] = 1 if c < r
    mask = np.tril(np.ones((128, 128), np.float32), k=-1).astype(NP_MM)
    in_maps = []
    for c in range(NCORES):
        b, k4 = divmod(c, 4)
        heads = [4 * k4 + i for i in range(HPC)]

        def pairw(kern):
            # [NPAIRS, MK, 128, 128] lhsT chunks
            out = np.empty((NPAIRS, MK, 128, 128), NP_MM)
            for pr in range(NPAIRS):
                pairm = np.concatenate(
                    [kern[heads[2 * pr]], kern[heads[2 * pr + 1]]], axis=1
                )  # [1024, 128]
                out[pr] = pairm.reshape(MK, 128, 128).astype(NP_MM)
            return out

        def pairb(bias):
            out = np.empty((NPAIRS, 1, 128), NP_MM)
            for pr in range(NPAIRS):
                out[pr, 0] = np.concatenate(
                    [bias[heads[2 * pr]], bias[heads[2 * pr + 1]]]
                ).astype(NP_MM)
            return out

        wo = np.empty((NPAIRS, 128, 1024), NP_MM)
        for pr in range(NPAIRS):
            wo[pr] = np.concatenate(
                [ko[heads[2 * pr]], ko[heads[2 * pr + 1]]], axis=0
            ).astype(NP_MM)

        in_maps.append({
            "xT": xT[b],
            "wq": pairw(kq), "wk": pairw(kk), "wv": pairw(kv),
            "wo": wo,
            "bq": pairb(bq), "bk": pairb(bk), "bv": pairb(bv),
            "eye": eye, "mask": mask,
        })
    return in_maps


def kernel(x, kernel_query, kernel_key, kernel_value, kernel_out,
           bias_query, bias_key, bias_value, bias_out, _trace=False):
    x = np.asarray(x, np.float32)
    kq = np.asarray(kernel_query, np.float32)
    kk = np.asarray(kernel_key, np.float32)
    kv = np.asarray(kernel_value, np.float32)
    ko = np.asarray(kernel_out, np.float32)
    bq = np.asarray(bias_query, np.float32)
    bk = np.asarray(bias_key, np.float32)
    bv = np.asarray(bias_value, np.float32)
    bo = np.asarray(bias_out, np.float32)

    nc = _build()
    in_maps = _host_inputs(x, kq, kk, kv, ko, bq, bk, bv)
    res = bass_utils.run_bass_kernel_spmd(
        nc, in_maps, core_ids=list(range(NCORES)), trace=_trace
    )
    out = np.zeros((B, P, M), np.float32)
    for c in range(NCORES):
        out[c // 4] += res.results[c]["outp"]
    out += bo[None, None, :]

    # patch fully-masked query row P-1: uniform attention = mean_k v
    for b in range(B):
        xbar = x[b].mean(axis=0, dtype=np.float64)  # [M]
        row = np.zeros(M, np.float64)
        for n in range(N):
            zrow = xbar @ kv[n].astype(np.float64) + bv[n].astype(np.float64)
            row += zrow @ ko[n].astype(np.float64)
        out[b, P - 1, :] = (row + bo.astype(np.float64)).astype(np.float32)

    if _trace:
        kernel._last_result = res
    return out


# revision 27
# speedup vs baseline: 1.5043x; 1.0858x over previous
"""Trainium2 Bass kernel for multi-head attention (B=2, P=2048, M=1024, N=16, H=64).

Sharding: 8 cores = 2 batches x 4 head-groups. Core c handles batch c//4,
heads [4*(c%4), 4*(c%4)+4). Each core computes its heads' attention and the
partial output projection; the host sums partials across the 4 cores of each
batch.

Device algorithm (per core; matmul dtype selectable bf16/fp32r):
  - q^T,k^T,v^T [h', p] via projections with x^T as the moving operand,
    head-pairs concatenated to fill 128 partitions; bias added via K=1 matmul.
    One weight load feeds 4 accumulating p-tiles (LDWEIGHTS amortized).
  - scores^T [pk, pq] per head; strictly-lower-triangular keep mask (pq < pk)
    exploited by skipping fully-masked tiles and narrowing partial ones.
    Two pk-chunks of scores land in one [128,1024] PSUM tile so a single
    ScalarE exp instruction covers both (amortizes ACT fixed overhead).
  - v transposed head-wise on the PE with an appended ones row, so the z
    matmul (z_aug^T = v_aug^T @ exp^T) also yields the softmax denominators.
  - z_aug^T is PE-transposed to [pq, h] layout where the denominator is a
    per-partition scalar: reciprocal + tensor_scalar normalize, then
    PE-transposed back and head-pairs packed to K=128 for the output
    projection, which accumulates both pairs in PSUM. This per-unit work is
    interleaved with the attention stream to keep the PE fed while ScalarE
    runs exp.
  - The fully-masked query row P-1 (softmax of all -1e10 = uniform) is
    patched analytically on the host.
"""
import os
import sys

import numpy as np

if "/opt/trn_rl_repo" not in sys.path:
    sys.path.insert(0, "/opt/trn_rl_repo")

import concourse.bacc as bacc
import concourse.tile as tile
from concourse import mybir
from concourse import bass_utils
import ml_dtypes

B, P, M, N, H = 2, 2048, 1024, 16, 64
NCORES = 8
HPC = 4          # heads per core
NPAIRS = 2       # head pairs per core
MK = M // 128    # 8 contraction chunks for projections
PT = P // 512    # 4 free-dim tiles of 512 over sequence
PC = P // 128    # 16 partition chunks over sequence
MT = M // 512    # 2 output m-tiles

F32 = mybir.dt.float32
F32R = mybir.dt.float32r
BF16 = mybir.dt.bfloat16
EXP = mybir.ActivationFunctionType.Exp
MULT = mybir.AluOpType.mult

DT_MODE = os.environ.get("KERNEL_DT", "bf16")   # "bf16" | "f32r"
DT_MM = BF16 if DT_MODE == "bf16" else F32R
NP_MM = ml_dtypes.bfloat16 if DT_MODE == "bf16" else np.float32

_BUILT = {}


def _emit(nc, tc, aps, ctx):
    xT = aps["xT"]          # [1024, 2048]
    outp = aps["outp"]      # [2048, 1024]

    consts = ctx.enter_context(tc.tile_pool(name="consts", bufs=1))
    xpool = ctx.enter_context(tc.tile_pool(name="xpool", bufs=MK))
    qkpool = ctx.enter_context(tc.tile_pool(name="qkpool", bufs=2))
    vapool = ctx.enter_context(tc.tile_pool(name="vapool", bufs=68))
    zppool = ctx.enter_context(tc.tile_pool(name="zppool", bufs=16))

    eye = consts.tile([128, 128], F32)
    nc.sync.dma_start(eye[:], aps["eye"][:])
    mask = consts.tile([128, 128], DT_MM)
    nc.sync.dma_start(mask[:], aps["mask"][:])
    ones32 = consts.tile([1, 512], F32)
    nc.vector.memset(ones32[:], 1.0)
    if DT_MODE == "bf16":
        ones_mm = consts.tile([1, 512], BF16)
        nc.vector.memset(ones_mm[:], 1.0)
    else:
        ones_mm = consts.tile([1, 512], F32R)
        nc.vector.tensor_copy(ones_mm[:], ones32[:])
    wos = []
    for pr in range(NPAIRS):
        wot = consts.tile([128, 1024], DT_MM, tag=f"wo{pr}", name=f"wo{pr}")
        nc.sync.dma_start(wot[:], aps["wo"][pr])
        wos.append(wot)

    # x^T chunks [128 m, 2048 p]
    xsb = []
    for k in range(MK):
        xt = xpool.tile([128, 2048], DT_MM, tag="x")
        nc.sync.dma_start(xt[:], xT[128 * k:128 * (k + 1), :])
        xsb.append(xt)

    tiles = {}
    qts, kts = {}, {}

    # ---------------- QKV projections ----------------
    with tc.tile_pool(name="wpool", bufs=6) as wpool, \
         tc.tile_pool(name="vtpool", bufs=4) as vtpool, \
         tc.tile_pool(name="ps_qkv", bufs=4, space="PSUM") as ps_qkv:
        wsb = {}
        bsb = {}
        for t in ("q", "k", "v"):
            for pr in range(NPAIRS):
                wt = wpool.tile([128, MK * 128], DT_MM, tag="w")
                nc.sync.dma_start(
                    wt.rearrange("p (k f) -> p k f", k=MK),
                    aps[f"w{t}"][pr].rearrange("k p f -> p k f"),
                )
                wsb[(t, pr)] = wt
                bt = consts.tile([1, 128], DT_MM, tag=f"b{t}{pr}")
                nc.sync.dma_start(bt[:], aps[f"b{t}"][pr])
                bsb[(t, pr)] = bt
        for pr in range(NPAIRS):
            qT = qkpool.tile([128, 2048], DT_MM, tag="qT", name=f"qT{pr}")
            kT = qkpool.tile([128, 2048], DT_MM, tag="kT", name=f"kT{pr}")
            qts[pr], kts[pr] = qT, kT
            for t, dest in (("q", qT), ("k", kT), ("v", None)):
                w = wsb[(t, pr)]
                # one LDWEIGHTS per m-chunk feeds 4 accumulating p-tiles
                pss = [ps_qkv.tile([128, 512], F32, tag="qkvps",
                                   name=f"qkvps_{t}{pr}{j4}")
                       for j4 in range(PT)]
                for mk in range(MK):
                    for j4 in range(PT):
                        nc.tensor.matmul(
                            pss[j4][:],
                            w[:, 128 * mk:128 * (mk + 1)],
                            xsb[mk][:, 512 * j4:512 * (j4 + 1)],
                            start=(mk == 0), stop=False,
                        )
                for j4 in range(PT):
                    nc.tensor.matmul(
                        pss[j4][:], bsb[(t, pr)][:],
                        ones_mm[:], start=False, stop=True,
                    )
                for j4 in range(PT):
                    ps = pss[j4]
                    if t == "v":
                        # v^T slice + ones row, PE-transposed into v_aug
                        # chunks [128 pk, 65] (col 64 = ones for denoms)
                        for h01 in range(2):
                            vts = vtpool.tile([65, 512], F32, tag="vT")
                            nc.vector.tensor_copy(vts[64:65, :], ones32[:])
                            nc.vector.tensor_copy(
                                vts[0:64, :], ps[64 * h01:64 * (h01 + 1), :]
                            )
                            for c4 in range(4):
                                pst = ps_qkv.tile([128, 65], F32,
                                                  tag="qkvps")
                                nc.tensor.transpose(
                                    pst[:], vts[:, 128 * c4:128 * (c4 + 1)],
                                    eye[0:65, 0:65],
                                )
                                va = vapool.tile([128, 65], DT_MM, tag="va")
                                nc.vector.tensor_copy(va[:], pst[:])
                                tiles[("va", pr, h01, 4 * j4 + c4)] = va
                    else:
                        nc.vector.tensor_copy(
                            dest[:, 512 * j4:512 * (j4 + 1)], ps[:]
                        )

    # ------- attention + normalization + projection, pipelined per j -------
    with tc.tile_pool(name="expool", bufs=9) as expool, \
         tc.tile_pool(name="zsbpool", bufs=6) as zsbpool, \
         tc.tile_pool(name="znpool", bufs=6) as znpool, \
         tc.tile_pool(name="rcpool", bufs=8) as rcpool, \
         tc.tile_pool(name="opool", bufs=4) as opool, \
         tc.tile_pool(name="ps_sc", bufs=2, space="PSUM") as ps_sc, \
         tc.tile_pool(name="ps_z", bufs=1, space="PSUM") as ps_z, \
         tc.tile_pool(name="ps_t", bufs=3, space="PSUM") as ps_t:
        ps_pr = ps_t
        for j in range(PT):
            for pr in range(NPAIRS):
                for c4 in range(4):
                    tiles[("zp", pr, 4 * j + c4)] = zppool.tile(
                        [128, 128], DT_MM, tag="zp",
                        name=f"zp{pr}_{4 * j + c4}")
            for u in range(HPC):
                pr, h01 = divmod(u, 2)
                qT, kT = qts[pr], kts[pr]
                rows = slice(64 * h01, 64 * (h01 + 1))
                zps = ps_z.tile([65, 512], F32, tag="zps")
                ilist = list(range(PC - 1, 4 * j - 1, -1))
                ipairs = [(ilist[a], ilist[a + 1])
                          for a in range(0, len(ilist), 2)]
                npair = len(ipairs)
                DW = 6      # z trails scores/exp by DW pairs (PE never
                            # blocks in-order on ScalarE's exp latency)
                descs = []
                for idx in range(npair + DW):
                    if idx < npair:
                        i1, i2 = ipairs[idx]
                        w1 = min(512, 128 * (i1 - 4 * j + 1))
                        w2 = min(512, 128 * (i2 - 4 * j + 1))
                        sps = ps_sc.tile([128, 1024], F32, tag="scps")
                        nc.tensor.matmul(
                            sps[:, :w1],
                            kT[rows, 128 * i1:128 * (i1 + 1)],
                            qT[rows, 512 * j:512 * j + w1],
                            start=True, stop=True,
                        )
                        nc.tensor.matmul(
                            sps[:, 512:512 + w2],
                            kT[rows, 128 * i2:128 * (i2 + 1)],
                            qT[rows, 512 * j:512 * j + w2],
                            start=True, stop=True,
                        )
                        ex = expool.tile([128, 1024], DT_MM, tag="ex")
                        if w1 == 512:
                            nc.scalar.activation(
                                ex[:, :512 + w2], sps[:, :512 + w2], EXP,
                                scale=0.125,
                            )
                        else:
                            nc.scalar.activation(
                                ex[:, :w1], sps[:, :w1], EXP, scale=0.125
                            )
                            nc.scalar.activation(
                                ex[:, 512:512 + w2], sps[:, 512:512 + w2],
                                EXP, scale=0.125,
                            )
                        for i_, off, w_ in ((i1, 0, w1), (i2, 512, w2)):
                            tt = i_ - 4 * j
                            if tt < 4:
                                nc.vector.tensor_mul(
                                    ex[:, off + 128 * tt:off + w_],
                                    ex[:, off + 128 * tt:off + w_], mask[:]
                                )
                        descs.append((ex, i1, i2, w1, w2))
                    zi = idx - DW
                    if 0 <= zi < npair:
                        ex, i1, i2, w1, w2 = descs[zi]
                        nc.tensor.matmul(
                            zps[:, :w1], tiles[("va", pr, h01, i1)][:],
                            ex[:, :w1],
                            start=(zi == 0), stop=False,
                        )
                        nc.tensor.matmul(
                            zps[:, :w2], tiles[("va", pr, h01, i2)][:],
                            ex[:, 512:512 + w2],
                            start=False, stop=(zi == npair - 1),
                        )
                zsb = zsbpool.tile([65, 512], F32, tag="z")
                nc.vector.tensor_copy(zsb[:], zps[:])
                if j == PT - 1:
                    # fully-masked query row P-1: denom 0 -> 1 so the
                    # reciprocal is finite (host patches the output row)
                    nc.vector.tensor_copy(zsb[64:65, 511:512], ones32[:, 0:1])
                # normalize in pq-space and pack into zpair
                for c4 in range(4):
                    pst1 = ps_t.tile([128, 65], F32, tag="tps")
                    nc.tensor.transpose(
                        pst1[:], zsb[:, 128 * c4:128 * (c4 + 1)],
                        eye[0:65, 0:65],
                    )
                    rcol = rcpool.tile([128, 1], F32, tag="rc")
                    nc.vector.reciprocal(rcol[:], pst1[:, 64:65])
                    zn = znpool.tile([128, 64], F32, tag="zn")
                    nc.vector.tensor_scalar_mul(zn[:], pst1[:, 0:64], rcol[:])
                    pst2 = ps_t.tile([64, 128], F32, tag="tps")
                    nc.tensor.transpose(pst2[:], zn[:], eye[:])
                    nc.vector.tensor_copy(
                        tiles[("zp", pr, 4 * j + c4)][rows, :], pst2[:]
                    )
            # projections for this j's pq chunks (both pairs accumulate)
            for c4 in range(4):
                ck = 4 * j + c4
                for mt in range(MT):
                    pp = ps_pr.tile([128, 512], F32, tag="tps", name=f"prps{ck}_{mt}")
                    nc.tensor.matmul(
                        pp[:], tiles[("zp", 0, ck)][:],
                        wos[0][:, 512 * mt:512 * (mt + 1)],
                        start=True, stop=False,
                    )
                    nc.tensor.matmul(
                        pp[:], tiles[("zp", 1, ck)][:],
                        wos[1][:, 512 * mt:512 * (mt + 1)],
                        start=False, stop=True,
                    )
                    osb = opool.tile([128, 512], F32, tag="osb")
                    nc.scalar.copy(osb[:], pp[:])
                    nc.sync.dma_start(
                        outp[128 * ck:128 * (ck + 1),
                             512 * mt:512 * (mt + 1)],
                        osb[:],
                    )


def _build():
    if DT_MODE in _BUILT:
        return _BUILT[DT_MODE]
    from contextlib import ExitStack

    nc = bacc.Bacc("TRN2", target_bir_lowering=False, debug=False)
    aps = {
        "xT": nc.dram_tensor("xT", [M, P], DT_MM, kind="ExternalInput").ap(),
        "wq": nc.dram_tensor("wq", [NPAIRS, MK, 128, 128], DT_MM,
                             kind="ExternalInput").ap(),
        "wk": nc.dram_tensor("wk", [NPAIRS, MK, 128, 128], DT_MM,
                             kind="ExternalInput").ap(),
        "wv": nc.dram_tensor("wv", [NPAIRS, MK, 128, 128], DT_MM,
                             kind="ExternalInput").ap(),
        "wo": nc.dram_tensor("wo", [NPAIRS, 128, 1024], DT_MM,
                             kind="ExternalInput").ap(),
        "bq": nc.dram_tensor("bq", [NPAIRS, 1, 128], DT_MM,
                             kind="ExternalInput").ap(),
        "bk": nc.dram_tensor("bk", [NPAIRS, 1, 128], DT_MM,
                             kind="ExternalInput").ap(),
        "bv": nc.dram_tensor("bv", [NPAIRS, 1, 128], DT_MM,
                             kind="ExternalInput").ap(),
        "eye": nc.dram_tensor("eye", [128, 128], F32,
                              kind="ExternalInput").ap(),
        "mask": nc.dram_tensor("mask", [128, 128], DT_MM,
                               kind="ExternalInput").ap(),
        "outp": nc.dram_tensor("outp", [P, M], F32, kind="ExternalOutput").ap(),
    }
    with tile.TileContext(nc) as tc:
        with ExitStack() as ctx:
            _emit(nc, tc, aps, ctx)
    nc.compile()
    _BUILT[DT_MODE] = nc
    return nc


def _host_inputs(x, kq, kk, kv, ko, bq, bk, bv):
    xT = np.ascontiguousarray(x.transpose(0, 2, 1)).astype(NP_MM)  # [B, M, P]
    eye = np.eye(128, dtype=np.float32)
    # keep iff pq < pk; block mask[r(pk), c(pq)] = 1 if c < r
    mask = np.tril(np.ones((128, 128), np.float32), k=-1).astype(NP_MM)
    in_maps = []
    for c in range(NCORES):
        b, k4 = divmod(c, 4)
        heads = [4 * k4 + i for i in range(HPC)]

        def pairw(kern):
            # [NPAIRS, MK, 128, 128] lhsT chunks
            out = np.empty((NPAIRS, MK, 128, 128), NP_MM)
            for pr in range(NPAIRS):
                pairm = np.concatenate(
                    [kern[heads[2 * pr]], kern[heads[2 * pr + 1]]], axis=1
                )  # [1024, 128]
                out[pr] = pairm.reshape(MK, 128, 128).astype(NP_MM)
            return out

        def pairb(bias):
            out = np.empty((NPAIRS, 1, 128), NP_MM)
            for pr in range(NPAIRS):
                out[pr, 0] = np.concatenate(
                    [bias[heads[2 * pr]], bias[heads[2 * pr + 1]]]
                ).astype(NP_MM)
            return out

        wo = np.empty((NPAIRS, 128, 1024), NP_MM)
        for pr in range(NPAIRS):
            wo[pr] = np.concatenate(
                [ko[heads[2 * pr]], ko[heads[2 * pr + 1]]], axis=0
            ).astype(NP_MM)

        in_maps.append({
            "xT": xT[b],
            "wq": pairw(kq), "wk": pairw(kk), "wv": pairw(kv),
            "wo": wo,
            "bq": pairb(bq), "bk": pairb(bk), "bv": pairb(bv),
            "eye": eye, "mask": mask,
        })
    return in_maps


def kernel(x, kernel_query, kernel_key, kernel_value, kernel_out,
           bias_query, bias_key, bias_value, bias_out, _trace=False):
    x = np.asarray(x, np.float32)
    kq = np.asarray(kernel_query, np.float32)
    kk = np.asarray(kernel_key, np.float32)
    kv = np.asarray(kernel_value, np.float32)
    ko = np.asarray(kernel_out, np.float32)
    bq = np.asarray(bias_query, np.float32)
    bk = np.asarray(bias_key, np.float32)
    bv = np.asarray(bias_value, np.float32)
    bo = np.asarray(bias_out, np.float32)

    nc = _build()
    in_maps = _host_inputs(x, kq, kk, kv, ko, bq, bk, bv)
    res = bass_utils.run_bass_kernel_spmd(
        nc, in_maps, core_ids=list(range(NCORES)), trace=_trace
    )
    out = np.zeros((B, P, M), np.float32)
    for c in range(NCORES):
        out[c // 4] += res.results[c]["outp"]
    out += bo[None, None, :]

    # patch fully-masked query row P-1: uniform attention = mean_k v
    for b in range(B):
        xbar = x[b].mean(axis=0, dtype=np.float64)  # [M]
        row = np.zeros(M, np.float64)
        for n in range(N):
            zrow = xbar @ kv[n].astype(np.float64) + bv[n].astype(np.float64)
            row += zrow @ ko[n].astype(np.float64)
        out[b, P - 1, :] = (row + bo.astype(np.float64)).astype(np.float32)

    if _trace:
        kernel._last_result = res
    return out


# revision 28
# speedup vs baseline: 1.5433x; 1.0259x over previous
"""Trainium2 Bass kernel for multi-head attention (B=2, P=2048, M=1024, N=16, H=64).

Sharding: 8 cores = 2 batches x 4 head-groups. Core c handles batch c//4,
heads [4*(c%4), 4*(c%4)+4). Each core computes its heads' attention and the
partial output projection; the host sums partials across the 4 cores of each
batch.

Device algorithm (per core; matmul dtype selectable bf16/fp32r):
  - q^T,k^T,v^T [h', p] via projections with x^T as the moving operand,
    head-pairs concatenated to fill 128 partitions; bias added via K=1 matmul.
    One weight load feeds 4 accumulating p-tiles (LDWEIGHTS amortized).
  - scores^T [pk, pq] per head; strictly-lower-triangular keep mask (pq < pk)
    exploited by skipping fully-masked tiles and narrowing partial ones.
    Two pk-chunks of scores land in one [128,1024] PSUM tile so a single
    ScalarE exp instruction covers both (amortizes ACT fixed overhead).
  - v transposed head-wise on the PE with an appended ones row, so the z
    matmul (z_aug^T = v_aug^T @ exp^T) also yields the softmax denominators.
  - z_aug^T is PE-transposed to [pq, h] layout where the denominator is a
    per-partition scalar: reciprocal + tensor_scalar normalize, then
    PE-transposed back and head-pairs packed to K=128 for the output
    projection, which accumulates both pairs in PSUM. This per-unit work is
    interleaved with the attention stream to keep the PE fed while ScalarE
    runs exp.
  - The fully-masked query row P-1 (softmax of all -1e10 = uniform) is
    patched analytically on the host.
"""
import os
import sys

import numpy as np

if "/opt/trn_rl_repo" not in sys.path:
    sys.path.insert(0, "/opt/trn_rl_repo")

import concourse.bacc as bacc
import concourse.tile as tile
from concourse import mybir
from concourse import bass_utils
import ml_dtypes

B, P, M, N, H = 2, 2048, 1024, 16, 64
NCORES = 8
HPC = 4          # heads per core
NPAIRS = 2       # head pairs per core
MK = M // 128    # 8 contraction chunks for projections
PT = P // 512    # 4 free-dim tiles of 512 over sequence
PC = P // 128    # 16 partition chunks over sequence
MT = M // 512    # 2 output m-tiles

F32 = mybir.dt.float32
F32R = mybir.dt.float32r
BF16 = mybir.dt.bfloat16
EXP = mybir.ActivationFunctionType.Exp
MULT = mybir.AluOpType.mult

DT_MODE = os.environ.get("KERNEL_DT", "bf16")   # "bf16" | "f32r"
DT_MM = BF16 if DT_MODE == "bf16" else F32R
NP_MM = ml_dtypes.bfloat16 if DT_MODE == "bf16" else np.float32

_BUILT = {}


def _emit(nc, tc, aps, ctx):
    xT = aps["xT"]          # [1024, 2048]
    outp = aps["outp"]      # [2048, 1024]

    consts = ctx.enter_context(tc.tile_pool(name="consts", bufs=1))
    xpool = ctx.enter_context(tc.tile_pool(name="xpool", bufs=MK))
    qkpool = ctx.enter_context(tc.tile_pool(name="qkpool", bufs=2))
    vapool = ctx.enter_context(tc.tile_pool(name="vapool", bufs=68))
    zppool = ctx.enter_context(tc.tile_pool(name="zppool", bufs=16))

    eye = consts.tile([128, 128], F32)
    nc.sync.dma_start(eye[:], aps["eye"][:])
    mask = consts.tile([128, 128], DT_MM)
    nc.sync.dma_start(mask[:], aps["mask"][:])
    ones32 = consts.tile([1, 512], F32)
    nc.vector.memset(ones32[:], 1.0)
    if DT_MODE == "bf16":
        ones_mm = consts.tile([1, 512], BF16)
        nc.vector.memset(ones_mm[:], 1.0)
    else:
        ones_mm = consts.tile([1, 512], F32R)
        nc.vector.tensor_copy(ones_mm[:], ones32[:])
    wos = []
    for pr in range(NPAIRS):
        wot = consts.tile([128, 1024], DT_MM, tag=f"wo{pr}", name=f"wo{pr}")
        nc.sync.dma_start(wot[:], aps["wo"][pr])
        wos.append(wot)

    # x^T chunks [128 m, 2048 p]
    xsb = []
    for k in range(MK):
        xt = xpool.tile([128, 2048], DT_MM, tag="x")
        nc.sync.dma_start(xt[:], xT[128 * k:128 * (k + 1), :])
        xsb.append(xt)

    tiles = {}
    qts, kts = {}, {}

    # ---------------- QKV projections ----------------
    with tc.tile_pool(name="wpool", bufs=6) as wpool, \
         tc.tile_pool(name="vtpool", bufs=4) as vtpool, \
         tc.tile_pool(name="ps_qkv", bufs=4, space="PSUM") as ps_qkv:
        wsb = {}
        bsb = {}
        for t in ("q", "k", "v"):
            for pr in range(NPAIRS):
                wt = wpool.tile([128, MK * 128], DT_MM, tag="w")
                nc.sync.dma_start(
                    wt.rearrange("p (k f) -> p k f", k=MK),
                    aps[f"w{t}"][pr].rearrange("k p f -> p k f"),
                )
                wsb[(t, pr)] = wt
                bt = consts.tile([1, 128], DT_MM, tag=f"b{t}{pr}")
                nc.sync.dma_start(bt[:], aps[f"b{t}"][pr])
                bsb[(t, pr)] = bt
        for pr in range(NPAIRS):
            qT = qkpool.tile([128, 2048], DT_MM, tag="qT", name=f"qT{pr}")
            kT = qkpool.tile([128, 2048], DT_MM, tag="kT", name=f"kT{pr}")
            qts[pr], kts[pr] = qT, kT
            for t, dest in (("q", qT), ("k", kT), ("v", None)):
                w = wsb[(t, pr)]
                # one LDWEIGHTS per m-chunk feeds 4 accumulating p-tiles
                pss = [ps_qkv.tile([128, 512], F32, tag="qkvps",
                                   name=f"qkvps_{t}{pr}{j4}")
                       for j4 in range(PT)]
                for mk in range(MK):
                    for j4 in range(PT):
                        nc.tensor.matmul(
                            pss[j4][:],
                            w[:, 128 * mk:128 * (mk + 1)],
                            xsb[mk][:, 512 * j4:512 * (j4 + 1)],
                            start=(mk == 0), stop=False,
                        )
                for j4 in range(PT):
                    nc.tensor.matmul(
                        pss[j4][:], bsb[(t, pr)][:],
                        ones_mm[:], start=False, stop=True,
                    )
                for j4 in range(PT):
                    ps = pss[j4]
                    if t == "v":
                        # v^T slice + ones row, PE-transposed into v_aug
                        # chunks [128 pk, 65] (col 64 = ones for denoms)
                        for h01 in range(2):
                            vts = vtpool.tile([65, 512], F32, tag="vT")
                            nc.vector.tensor_copy(vts[64:65, :], ones32[:])
                            nc.vector.tensor_copy(
                                vts[0:64, :], ps[64 * h01:64 * (h01 + 1), :]
                            )
                            for c4 in range(4):
                                pst = ps_qkv.tile([128, 65], F32,
                                                  tag="qkvps")
                                nc.tensor.transpose(
                                    pst[:], vts[:, 128 * c4:128 * (c4 + 1)],
                                    eye[0:65, 0:65],
                                )
                                va = vapool.tile([128, 65], DT_MM, tag="va")
                                nc.vector.tensor_copy(va[:], pst[:])
                                tiles[("va", pr, h01, 4 * j4 + c4)] = va
                    else:
                        nc.vector.tensor_copy(
                            dest[:, 512 * j4:512 * (j4 + 1)], ps[:]
                        )

    # ------- attention + normalization + projection, pipelined per j -------
    with tc.tile_pool(name="expool", bufs=9) as expool, \
         tc.tile_pool(name="zsbpool", bufs=6) as zsbpool, \
         tc.tile_pool(name="znpool", bufs=6) as znpool, \
         tc.tile_pool(name="rcpool", bufs=8) as rcpool, \
         tc.tile_pool(name="opool", bufs=4) as opool, \
         tc.tile_pool(name="ps_sc", bufs=2, space="PSUM") as ps_sc, \
         tc.tile_pool(name="ps_z", bufs=1, space="PSUM") as ps_z, \
         tc.tile_pool(name="ps_t", bufs=3, space="PSUM") as ps_t:
        ps_pr = ps_t
        for j in range(PT):
            for pr in range(NPAIRS):
                for c4 in range(4):
                    tiles[("zp", pr, 4 * j + c4)] = zppool.tile(
                        [128, 128], DT_MM, tag="zp",
                        name=f"zp{pr}_{4 * j + c4}")
            for pr in range(NPAIRS):
                qT, kT = qts[pr], kts[pr]
                ilist = list(range(PC - 1, 4 * j - 1, -1))
                ipairs = [(ilist[a], ilist[a + 1])
                          for a in range(0, len(ilist), 2)]
                npair = len(ipairs)
                # merged round-robin stream over both heads of the pair:
                # slot s -> (head s%2, chunk-pair s//2); z matmuls trail by
                # DW slots so the PE never blocks in-order on exp latency
                nslot = 2 * npair
                DW = min(6, nslot - 1)
                zpss, descs = [], []
                for h01 in range(2):
                    zpss.append(ps_z.tile([65, 512], F32, tag="zps",
                                          name=f"zps{pr}_{h01}_{j}"))
                for idx in range(nslot + DW):
                    if idx < nslot:
                        h01, a = idx % 2, idx // 2
                        rows = slice(64 * h01, 64 * (h01 + 1))
                        i1, i2 = ipairs[a]
                        w1 = min(512, 128 * (i1 - 4 * j + 1))
                        w2 = min(512, 128 * (i2 - 4 * j + 1))
                        sps = ps_sc.tile([128, 1024], F32, tag="scps")
                        nc.tensor.matmul(
                            sps[:, :w1],
                            kT[rows, 128 * i1:128 * (i1 + 1)],
                            qT[rows, 512 * j:512 * j + w1],
                            start=True, stop=True,
                        )
                        nc.tensor.matmul(
                            sps[:, 512:512 + w2],
                            kT[rows, 128 * i2:128 * (i2 + 1)],
                            qT[rows, 512 * j:512 * j + w2],
                            start=True, stop=True,
                        )
                        ex = expool.tile([128, 1024], DT_MM, tag="ex")
                        if w1 == 512:
                            nc.scalar.activation(
                                ex[:, :512 + w2], sps[:, :512 + w2], EXP,
                                scale=0.125,
                            )
                        else:
                            nc.scalar.activation(
                                ex[:, :w1], sps[:, :w1], EXP, scale=0.125
                            )
                            nc.scalar.activation(
                                ex[:, 512:512 + w2], sps[:, 512:512 + w2],
                                EXP, scale=0.125,
                            )
                        for i_, off, w_ in ((i1, 0, w1), (i2, 512, w2)):
                            tt = i_ - 4 * j
                            if tt < 4:
                                nc.vector.tensor_mul(
                                    ex[:, off + 128 * tt:off + w_],
                                    ex[:, off + 128 * tt:off + w_], mask[:]
                                )
                        descs.append((ex, h01, i1, i2, w1, w2))
                    zi = idx - DW
                    if 0 <= zi < nslot:
                        ex, h01, i1, i2, w1, w2 = descs[zi]
                        nc.tensor.matmul(
                            zpss[h01][:, :w1],
                            tiles[("va", pr, h01, i1)][:], ex[:, :w1],
                            start=(zi < 2), stop=False,
                        )
                        nc.tensor.matmul(
                            zpss[h01][:, :w2],
                            tiles[("va", pr, h01, i2)][:],
                            ex[:, 512:512 + w2],
                            start=False, stop=(zi >= nslot - 2),
                        )
                for h01 in range(2):
                    rows = slice(64 * h01, 64 * (h01 + 1))
                    zsb = zsbpool.tile([65, 512], F32, tag="z",
                                       name=f"zsb{pr}_{h01}_{j}")
                    nc.vector.tensor_copy(zsb[:], zpss[h01][:])
                    if j == PT - 1:
                        # fully-masked query row P-1: denom 0 -> 1 so the
                        # reciprocal is finite (host patches the output row)
                        nc.vector.tensor_copy(
                            zsb[64:65, 511:512], ones32[:, 0:1]
                        )
                    # normalize in pq-space and pack into zpair
                    for c4 in range(4):
                        pst1 = ps_t.tile([128, 65], F32, tag="tps")
                        nc.tensor.transpose(
                            pst1[:], zsb[:, 128 * c4:128 * (c4 + 1)],
                            eye[0:65, 0:65],
                        )
                        rcol = rcpool.tile([128, 1], F32, tag="rc")
                        nc.vector.reciprocal(rcol[:], pst1[:, 64:65])
                        zn = znpool.tile([128, 64], F32, tag="zn")
                        nc.vector.tensor_scalar_mul(
                            zn[:], pst1[:, 0:64], rcol[:]
                        )
                        pst2 = ps_t.tile([64, 128], F32, tag="tps")
                        nc.tensor.transpose(pst2[:], zn[:], eye[:])
                        nc.vector.tensor_copy(
                            tiles[("zp", pr, 4 * j + c4)][rows, :], pst2[:]
                        )
            # projections for this j's pq chunks (both pairs accumulate)
            for c4 in range(4):
                ck = 4 * j + c4
                for mt in range(MT):
                    pp = ps_pr.tile([128, 512], F32, tag="tps", name=f"prps{ck}_{mt}")
                    nc.tensor.matmul(
                        pp[:], tiles[("zp", 0, ck)][:],
                        wos[0][:, 512 * mt:512 * (mt + 1)],
                        start=True, stop=False,
                    )
                    nc.tensor.matmul(
                        pp[:], tiles[("zp", 1, ck)][:],
                        wos[1][:, 512 * mt:512 * (mt + 1)],
                        start=False, stop=True,
                    )
                    osb = opool.tile([128, 512], F32, tag="osb")
                    nc.scalar.copy(osb[:], pp[:])
                    nc.sync.dma_start(
                        outp[128 * ck:128 * (ck + 1),
                             512 * mt:512 * (mt + 1)],
                        osb[:],
                    )


def _build():
    if DT_MODE in _BUILT:
        return _BUILT[DT_MODE]
    from contextlib import ExitStack

    nc = bacc.Bacc("TRN2", target_bir_lowering=False, debug=False)
    aps = {
        "xT": nc.dram_tensor("xT", [M, P], DT_MM, kind="ExternalInput").ap(),
        "wq": nc.dram_tensor("wq", [NPAIRS, MK, 128, 128], DT_MM,
                             kind="ExternalInput").ap(),
        "wk": nc.dram_tensor("wk", [NPAIRS, MK, 128, 128], DT_MM,
                             kind="ExternalInput").ap(),
        "wv": nc.dram_tensor("wv", [NPAIRS, MK, 128, 128], DT_MM,
                             kind="ExternalInput").ap(),
        "wo": nc.dram_tensor("wo", [NPAIRS, 128, 1024], DT_MM,
                             kind="ExternalInput").ap(),
        "bq": nc.dram_tensor("bq", [NPAIRS, 1, 128], DT_MM,
                             kind="ExternalInput").ap(),
        "bk": nc.dram_tensor("bk", [NPAIRS, 1, 128], DT_MM,
                             kind="ExternalInput").ap(),
        "bv": nc.dram_tensor("bv", [NPAIRS, 1, 128], DT_MM,
                             kind="ExternalInput").ap(),
        "eye": nc.dram_tensor("eye", [128, 128], F32,
                              kind="ExternalInput").ap(),
        "mask": nc.dram_tensor("mask", [128, 128], DT_MM,
                               kind="ExternalInput").ap(),
        "outp": nc.dram_tensor("outp", [P, M], F32, kind="ExternalOutput").ap(),
    }
    with tile.TileContext(nc) as tc:
        with ExitStack() as ctx:
            _emit(nc, tc, aps, ctx)
    nc.compile()
    _BUILT[DT_MODE] = nc
    return nc


def _host_inputs(x, kq, kk, kv, ko, bq, bk, bv):
    xT = np.ascontiguousarray(x.transpose(0, 2, 1)).astype(NP_MM)  # [B, M, P]
    eye = np.eye(128, dtype=np.float32)
    # keep iff pq < pk; block mask[r(pk), c(pq)] = 1 if c < r
    mask = np.tril(np.ones((128, 128), np.float32), k=-1).astype(NP_MM)
    in_maps = []
    for c in range(NCORES):
        b, k4 = divmod(c, 4)
        heads = [4 * k4 + i for i in range(HPC)]

        def pairw(kern):
            # [NPAIRS, MK, 128, 128] lhsT chunks
            out = np.empty((NPAIRS, MK, 128, 128), NP_MM)
            for pr in range(NPAIRS):
                pairm = np.concatenate(
                    [kern[heads[2 * pr]], kern[heads[2 * pr + 1]]], axis=1
                )  # [1024, 128]
                out[pr] = pairm.reshape(MK, 128, 128).astype(NP_MM)
            return out

        def pairb(bias):
            out = np.empty((NPAIRS, 1, 128), NP_MM)
            for pr in range(NPAIRS):
                out[pr, 0] = np.concatenate(
                    [bias[heads[2 * pr]], bias[heads[2 * pr + 1]]]
                ).astype(NP_MM)
            return out

        wo = np.empty((NPAIRS, 128, 1024), NP_MM)
        for pr in range(NPAIRS):
            wo[pr] = np.concatenate(
                [ko[heads[2 * pr]], ko[heads[2 * pr + 1]]], axis=0
            ).astype(NP_MM)

        in_maps.append({
            "xT": xT[b],
            "wq": pairw(kq), "wk": pairw(kk), "wv": pairw(kv),
            "wo": wo,
            "bq": pairb(bq), "bk": pairb(bk), "bv": pairb(bv),
            "eye": eye, "mask": mask,
        })
    return in_maps


def kernel(x, kernel_query, kernel_key, kernel_value, kernel_out,
           bias_query, bias_key, bias_value, bias_out, _trace=False):
    x = np.asarray(x, np.float32)
    kq = np.asarray(kernel_query, np.float32)
    kk = np.asarray(kernel_key, np.float32)
    kv = np.asarray(kernel_value, np.float32)
    ko = np.asarray(kernel_out, np.float32)
    bq = np.asarray(bias_query, np.float32)
    bk = np.asarray(bias_key, np.float32)
    bv = np.asarray(bias_value, np.float32)
    bo = np.asarray(bias_out, np.float32)

    nc = _build()
    in_maps = _host_inputs(x, kq, kk, kv, ko, bq, bk, bv)
    res = bass_utils.run_bass_kernel_spmd(
        nc, in_maps, core_ids=list(range(NCORES)), trace=_trace
    )
    out = np.zeros((B, P, M), np.float32)
    for c in range(NCORES):
        out[c // 4] += res.results[c]["outp"]
    out += bo[None, None, :]

    # patch fully-masked query row P-1: uniform attention = mean_k v
    for b in range(B):
        xbar = x[b].mean(axis=0, dtype=np.float64)  # [M]
        row = np.zeros(M, np.float64)
        for n in range(N):
            zrow = xbar @ kv[n].astype(np.float64) + bv[n].astype(np.float64)
            row += zrow @ ko[n].astype(np.float64)
        out[b, P - 1, :] = (row + bo.astype(np.float64)).astype(np.float32)

    if _trace:
        kernel._last_result = res
    return out
